# revision 4
# baseline (speedup 1.0000x reference)
"""KVMemory kernel for Trainium2 (8 NeuronCores, Bass/Tile).

Data-parallel over batch: each of the 8 cores handles 512 examples.

Gather strategy: the SWDGE vector-indirect DMA (InstDMACopy) only supports
one index per partition per instruction (~1us descriptor-gen each on Pool —
the old bottleneck at 400 instructions/core). Instead we use the custom
GPSIMD `dma_gather` (InstDMAGatherAnt, mlp library): int16 indices laid out
[128, NI/16] (idx i at partition i%16, col i//16, replicated to all 8
16-partition Q7 groups), gathering up to CK*128 rows per instruction into
[128, CK, 256] fp16 tiles.

int16 only addresses 32767 rows, so the 100K-row tables are COMPACTED per
core on the host: each core references at most 512*50=25600 < 32768 unique
rows; we upload table[unique_rows] per core and remap indices. Tables are
cast to fp16 (tolerance 2e-2; fp16 keeps rel err ~1e-3), halving gather
traffic and enabling the DVE 4x perf mode.

Examples are sorted by pair_length DESCENDING within each core (host perm,
inverted on output). Valid keys are always a prefix (mask = arange(K) <
pair_length), so tile t only gathers/computes K_ts[t] = max pair_length in
that tile columns — ~35% less traffic and DVE work. K_ts and compact-table
sizes are data-dependent; programs are compiled per (k_ts, nu_k, nu_v) and
cached.

Per 128-example tile:
  - chunked dma_gather of key/value embedding rows (fp16)
  - logits via fused multiply+row-reduce (scalar_tensor_tensor accum_out,
    fp16 in / fp32 accum) on DVE
  - masked softmax (host-precomputed -1e30 bias add -> reduce_max negate ->
    ACT Exp with accum row-sum); probs left UNNORMALIZED, 1/sumexp applied
    once to the final [128, 256] accumulator
  - weighted value sum on DVE via 4 interleaved scalar_tensor_tensor
    ping-pong chains, combined pairwise
"""

import sys

if "/opt/trn_rl_repo" not in sys.path:
    sys.path.insert(0, "/opt/trn_rl_repo")

import numpy as np

import concourse.mybir as mybir
import concourse.tile as tile
from concourse import bacc
import concourse.bass_utils as bass_utils

N_CORES = 8
B = 4096
K = 50
D = 256
BC = B // N_CORES          # examples per core
P = 128                    # partition tile (examples per tile)
NTILES = BC // P           # 4
NCHAIN = 4                 # parallel value-accumulation chains
CK = 50                    # max key-columns per dma_gather instruction
SINGLE_PACKET = False
SCRATCH = 16384            # dynamic_dma_scratch_size (SWDGE ring bytes)
ROW_PAD = 1024             # compact-table row-count quantum (program cache)
MASK_NEG = np.float32(-1e30)

_PROGRAM_CACHE = {}


def _build_program(k_ts, nu_k, nu_v):
    f32 = mybir.dt.float32
    f16 = mybir.dt.float16
    i16 = mybir.dt.int16
    nc = bacc.Bacc("TRN2", target_bir_lowering=False, debug=False,
                   num_devices=N_CORES, dynamic_dma_scratch_size=SCRATCH)

    sum_k = sum(k_ts)
    kidx_d = nc.dram_tensor("kidx", [P, 8 * sum_k], i16, kind="ExternalInput")
    vidx_d = nc.dram_tensor("vidx", [P, 8 * sum_k], i16, kind="ExternalInput")
    bias_d = nc.dram_tensor("bias", [BC, K], f32, kind="ExternalInput")
    query_d = nc.dram_tensor("query", [BC, D], f16, kind="ExternalInput")
    ktab_d = nc.dram_tensor("key_table", [nu_k, D], f16, kind="ExternalInput")
    vtab_d = nc.dram_tensor("value_table", [nu_v, D], f16, kind="ExternalInput")
    out_d = nc.dram_tensor("out", [BC, D], f16, kind="ExternalOutput")

    with tile.TileContext(nc) as tc:
        with (
            tc.tile_pool(name="emb", bufs=4) as emb_pool,
            tc.tile_pool(name="small", bufs=2) as sp,
        ):
            off = 0
            for t in range(NTILES):
                kt = int(k_ts[t])
                rows = slice(t * P, (t + 1) * P)

                kidx_t = sp.tile([P, 8 * kt], i16, tag="kidx", name=f"kidx_{t}")
                nc.sync.dma_start(out=kidx_t[:], in_=kidx_d[:, off:off + 8 * kt])
                vidx_t = sp.tile([P, 8 * kt], i16, tag="vidx", name=f"vidx_{t}")
                nc.sync.dma_start(out=vidx_t[:], in_=vidx_d[:, off:off + 8 * kt])
                off += 8 * kt
                bias_t = sp.tile([P, kt], f32, tag="bias", name=f"bias_{t}")
                nc.sync.dma_start(out=bias_t[:], in_=bias_d[rows, :kt])
                q_t = sp.tile([P, D], f16, tag="q", name=f"q_{t}")
                nc.sync.dma_start(out=q_t[:], in_=query_d[rows, :])

                kemb = emb_pool.tile([P, kt, D], f16, tag="emb", name=f"kemb_{t}")
                vemb = emb_pool.tile([P, kt, D], f16, tag="emb", name=f"vemb_{t}")
                for tab_d, idx_t, emb in ((ktab_d, kidx_t, kemb),
                                          (vtab_d, vidx_t, vemb)):
                    for c0 in range(0, kt, CK):
                        ckc = min(CK, kt - c0)
                        ni = P * ckc
                        nc.gpsimd.dma_gather(
                            emb[:, c0:c0 + ckc, :], tab_d[:],
                            idx_t[:, 8 * c0:8 * (c0 + ckc)], ni, ni, D,
                            single_packet=SINGLE_PACKET,
                        )

                # logits[p, k] = sum_d q[p, d] * kemb[p, k, d]   (fp32 accum)
                logits_t = sp.tile([P, kt], f32, tag="logits", name=f"logits_{t}")
                scratch = sp.tile([P, D], f16, tag="scratch", name=f"scratch_{t}")
                for k in range(kt):
                    nc.vector.scalar_tensor_tensor(
                        out=scratch[:],
                        in0=kemb[:, k, :],
                        scalar=1.0,
                        in1=q_t[:],
                        op0=mybir.AluOpType.bypass,
                        op1=mybir.AluOpType.mult,
                        accum_out=logits_t[:, k:k + 1],
                    )
                nc.vector.tensor_tensor(
                    out=logits_t[:], in0=logits_t[:], in1=bias_t[:],
                    op=mybir.AluOpType.add,
                )

                neg_max = sp.tile([P, 1], f32, tag="neg_max", name=f"neg_max_{t}")
                nc.vector.tensor_reduce(
                    out=neg_max[:], in_=logits_t[:],
                    axis=mybir.AxisListType.X, op=mybir.AluOpType.max, negate=True,
                )
                probs = sp.tile([P, kt], f32, tag="probs", name=f"probs_{t}")
                sumexp = sp.tile([P, 1], f32, tag="sumexp", name=f"sumexp_{t}")
                nc.scalar.activation(
                    out=probs[:], in_=logits_t[:],
                    func=mybir.ActivationFunctionType.Exp,
                    bias=neg_max[:, :1], scale=1.0,
                    accum_out=sumexp[:],
                )
                inv = sp.tile([P, 1], f32, tag="inv", name=f"inv_{t}")
                nc.vector.reciprocal(out=inv[:], in_=sumexp[:])

                # knowledge[p, d] = inv * sum_k probs[p, k] * vemb[p, k, d]
                nchain = min(NCHAIN, kt)
                bufs = [
                    [sp.tile([P, D], f16, tag=f"acc_{i}_{s}",
                             name=f"acc_{t}_{i}_{s}") for s in range(2)]
                    for i in range(nchain)
                ]
                heads = []
                for i in range(nchain):
                    nc.vector.tensor_scalar(
                        out=bufs[i][0][:], in0=vemb[:, i, :],
                        scalar1=probs[:, i:i + 1], scalar2=None,
                        op0=mybir.AluOpType.mult,
                    )
                    steps = 0
                    for k in range(i + nchain, kt, nchain):
                        cur, prev = bufs[i][(steps + 1) % 2], bufs[i][steps % 2]
                        nc.vector.scalar_tensor_tensor(
                            out=cur[:],
                            in0=vemb[:, k, :],
                            scalar=probs[:, k:k + 1],
                            in1=prev[:],
                            op0=mybir.AluOpType.mult,
                            op1=mybir.AluOpType.add,
                        )
                        steps += 1
                    heads.append(bufs[i][steps % 2])

                while len(heads) > 1:
                    nxt = []
                    for i in range(0, len(heads) - 1, 2):
                        nc.vector.tensor_tensor(
                            out=heads[i][:], in0=heads[i][:], in1=heads[i + 1][:],
                            op=mybir.AluOpType.add,
                        )
                        nxt.append(heads[i])
                    if len(heads) % 2:
                        nxt.append(heads[-1])
                    heads = nxt

                out_t = sp.tile([P, D], f16, tag="out", name=f"out_{t}")
                nc.vector.tensor_scalar(
                    out=out_t[:], in0=heads[0][:], scalar1=inv[:, :1],
                    scalar2=None, op0=mybir.AluOpType.mult,
                )
                nc.sync.dma_start(out=out_d[rows, :], in_=out_t[:])

    nc.compile()
    return nc


def _get_program(k_ts, nu_k, nu_v):
    key = (tuple(int(k) for k in k_ts), int(nu_k), int(nu_v))
    if key not in _PROGRAM_CACHE:
        _PROGRAM_CACHE[key] = _build_program(*key)
    return _PROGRAM_CACHE[key]


def _wrap_idx(lin):
    """lin[i] (int16) -> [128, NI/16]: idx i at partition i%16, col i//16,
    replicated to the 8 Q7 16-partition groups."""
    ni = lin.size
    a = lin.reshape(ni // 16, 16).T
    return np.ascontiguousarray(np.tile(a, (8, 1)))


def _compact_idx_blocks(ids_sorted, k_ts):
    """ids_sorted: [BC, K] table-row ids (sorted example order). Returns
    (uniq_rows, idx_blocks [128, 8*sum(k_ts)] int16) gathering only the
    j < k_ts[t] prefix columns of each tile."""
    gathered = []
    for t in range(NTILES):
        gathered.append(ids_sorted[t * P:(t + 1) * P, :k_ts[t]])
    flat = np.concatenate([g.ravel() for g in gathered])
    uniq, inv = np.unique(flat, return_inverse=True)
    assert uniq.size < 32768
    inv = inv.astype(np.int16)
    blocks = []
    pos = 0
    for t in range(NTILES):
        kt = k_ts[t]
        tile_inv = inv[pos:pos + P * kt].reshape(P, kt)
        pos += P * kt
        lin = np.ascontiguousarray(tile_inv.T).ravel()  # lin[k*128+p]
        blocks.append(_wrap_idx(lin))
    return uniq, np.concatenate(blocks, axis=1)


def _prepare(keys, values, pair_length, query, key_table, value_table):
    keys = np.asarray(keys).astype(np.int64)
    values = np.asarray(values).astype(np.int64)
    pair_length = np.asarray(pair_length).astype(np.int32)
    query = np.asarray(query, dtype=np.float32)
    ktab16 = np.asarray(key_table, dtype=np.float16)
    vtab16 = np.asarray(value_table, dtype=np.float16)

    # shared per-tile column counts (max over cores so one SPMD program)
    tile_maxes = np.zeros((N_CORES, NTILES), dtype=np.int64)
    perms = []
    for c in range(N_CORES):
        pl = pair_length[c * BC:(c + 1) * BC]
        perm = np.argsort(-pl, kind="stable")
        perms.append(perm)
        pl_s = pl[perm]
        tile_maxes[c] = [pl_s[t * P:(t + 1) * P].max() for t in range(NTILES)]
    k_ts = tuple(int(v) for v in tile_maxes.max(axis=0))

    per_core = []
    inv_perms = []
    nu_k_max = nu_v_max = 0
    for c in range(N_CORES):
        rows = slice(c * BC, (c + 1) * BC)
        perm = perms[c]
        inv = np.empty_like(perm)
        inv[perm] = np.arange(BC)
        inv_perms.append(inv)
        pl_s = pair_length[rows][perm]
        bias = np.where(
            np.arange(K, dtype=np.int32)[None, :] < pl_s[:, None],
            np.float32(0.0), MASK_NEG).astype(np.float32)

        uniq_k, kidx = _compact_idx_blocks(keys[rows][perm], k_ts)
        uniq_v, vidx = _compact_idx_blocks(values[rows][perm], k_ts)
        nu_k_max = max(nu_k_max, uniq_k.size)
        nu_v_max = max(nu_v_max, uniq_v.size)
        per_core.append({
            "kidx": kidx,
            "vidx": vidx,
            "bias": bias,
            "query": np.ascontiguousarray(query[rows][perm]).astype(np.float16),
            "_uniq_k": uniq_k,
            "_uniq_v": uniq_v,
        })

    nu_k = -(-nu_k_max // ROW_PAD) * ROW_PAD
    nu_v = -(-nu_v_max // ROW_PAD) * ROW_PAD
    for c in range(N_CORES):
        m = per_core[c]
        uk, uv = m.pop("_uniq_k"), m.pop("_uniq_v")
        kt_c = np.zeros((nu_k, D), dtype=np.float16)
        kt_c[:uk.size] = ktab16[uk]
        vt_c = np.zeros((nu_v, D), dtype=np.float16)
        vt_c[:uv.size] = vtab16[uv]
        m["key_table"] = kt_c
        m["value_table"] = vt_c
    return per_core, inv_perms, k_ts, nu_k, nu_v


def kernel(keys, values, pair_length, query, key_table, value_table):
    per_core, inv_perms, k_ts, nu_k, nu_v = _prepare(
        keys, values, pair_length, query, key_table, value_table)
    nc = _get_program(k_ts, nu_k, nu_v)
    res = bass_utils.run_bass_kernel_spmd(nc, per_core,
                                          core_ids=list(range(N_CORES)))
    out = np.concatenate(
        [res.results[c]["out"][inv_perms[c]] for c in range(N_CORES)], axis=0)
    return out.astype(np.float32)


# revision 18
# speedup vs baseline: 1.3509x; 1.3509x over previous
"""KVMemory kernel for Trainium2 (8 NeuronCores, Bass/Tile).

Data-parallel over batch: each of the 8 cores handles 512 examples.

Gather strategy: the SWDGE vector-indirect DMA (InstDMACopy) only supports
one index per partition per instruction (~1us descriptor-gen each on Pool —
the old bottleneck at 400 instructions/core). Instead we use the custom
GPSIMD `dma_gather` (InstDMAGatherAnt, mlp library): int16 indices laid out
[128, NI/16] (idx i at partition i%16, col i//16, replicated to all 8
16-partition Q7 groups), gathering up to CK*128 rows per instruction into
[128, CK, 256] fp16 tiles.

int16 only addresses 32767 rows, so the 100K-row tables are COMPACTED per
core on the host: each core references at most 512*50=25600 < 32768 unique
rows; we upload table[unique_rows] per core and remap indices. Tables are
cast to fp16 (tolerance 2e-2; fp16 keeps rel err ~1e-3), halving gather
traffic and enabling the DVE 4x perf mode.

Examples are sorted by pair_length DESCENDING within each core (host perm,
inverted on output). Valid keys are always a prefix (mask = arange(K) <
pair_length), so tile t only gathers/computes K_ts[t] = max pair_length in
that tile columns — ~35% less traffic and DVE work. K_ts and compact-table
sizes are data-dependent; programs are compiled per (k_ts, nu_k, nu_v) and
cached.

Per 128-example tile (engine balance: the fused scalar_tensor_tensor path
runs at 1x on DVE — is_scalar_tensor_tensor disables all DVE perf modes — so
the hot ops are spread across DVE (2x/4x-capable instructions), ACT, and the
Pool/GPSIMD vector path; emission is software-pipelined so tile t+1's
gathers run ahead of tile t's compute):
  - chunked dma_gather of key/value embedding rows (fp16)
  - logits: bulk products kemb*q (tensor_tensor, fp16 2x mode, q broadcast
    on the middle dim keeps the last dim packed) on DVE; per-k row-sums
    split between ACT activation(Copy, accum_out) (pays the 187ns
    read-accumulator tax per op) and one bulk gpsimd.tensor_reduce on Pool
  - masked softmax (host-precomputed -1e30 bias add -> reduce_max negate ->
    ACT Exp with accum row-sum); probs left UNNORMALIZED, 1/sumexp applied
    once to the final [128, 256] accumulator
  - value sum: per-k probs_k*vemb_k split between DVE tensor_scalar (fp16
    4x mode, f32 [128,1] scalar operand exempt) and ACT activation(Copy,
    scale=probs_k), then an in-place pairwise tree reduction over k via
    tensor_tensor adds (fp16 2x) on DVE; the products buffer is shared
    between the logits and value phases of a tile
"""

import sys

if "/opt/trn_rl_repo" not in sys.path:
    sys.path.insert(0, "/opt/trn_rl_repo")

import numpy as np

import concourse.mybir as mybir
import concourse.tile as tile
from concourse import bacc
import concourse.bass_utils as bass_utils

N_CORES = 8
B = 4096
K = 50
D = 256
BC = B // N_CORES          # examples per core
P = 128                    # partition tile (examples per tile)
NTILES = BC // P           # 4
CK = 50                    # max key-columns per dma_gather instruction
PCH = 8                    # key-columns per bulk-products tensor_tensor op
SINGLE_PACKET = False
SCRATCH = 16384            # dynamic_dma_scratch_size (SWDGE ring bytes)
ROW_PAD = 1024             # compact-table row-count quantum (program cache)
MASK_NEG = np.float32(-1e30)

_PROGRAM_CACHE = {}


def _build_program(k_ts, nu_k, nu_v):
    f32 = mybir.dt.float32
    f16 = mybir.dt.float16
    i16 = mybir.dt.int16
    nc = bacc.Bacc("TRN2", target_bir_lowering=False, debug=False,
                   num_devices=N_CORES, dynamic_dma_scratch_size=SCRATCH)

    sum_k = sum(k_ts)
    kidx_d = nc.dram_tensor("kidx", [P, 8 * sum_k], i16, kind="ExternalInput")
    vidx_d = nc.dram_tensor("vidx", [P, 8 * sum_k], i16, kind="ExternalInput")
    bias_d = nc.dram_tensor("bias", [BC, K], f32, kind="ExternalInput")
    query_d = nc.dram_tensor("query", [BC, D], f16, kind="ExternalInput")
    ktab_d = nc.dram_tensor("key_table", [nu_k, D], f16, kind="ExternalInput")
    vtab_d = nc.dram_tensor("value_table", [nu_v, D], f16, kind="ExternalInput")
    out_d = nc.dram_tensor("out", [BC, D], f16, kind="ExternalOutput")

    with tile.TileContext(nc) as tc:
        with (
            tc.tile_pool(name="kemb", bufs=2) as kemb_pool,
            tc.tile_pool(name="vemb", bufs=3) as vemb_pool,
            tc.tile_pool(name="pk", bufs=2) as pk_pool,
            tc.tile_pool(name="prod", bufs=2) as prod_pool,
            tc.tile_pool(name="pact", bufs=1) as pact_pool,
            tc.tile_pool(name="small", bufs=3) as sp,
        ):
            offs = [8 * sum(int(k) for k in k_ts[:t]) for t in range(NTILES)]
            stage = {}

            def emit_load(t):
                kt = int(k_ts[t])
                rows = slice(t * P, (t + 1) * P)
                off = offs[t]

                kidx_t = sp.tile([P, 8 * kt], i16, tag="kidx", name=f"kidx_{t}")
                nc.sync.dma_start(out=kidx_t[:], in_=kidx_d[:, off:off + 8 * kt])
                bias_t = sp.tile([P, kt], f32, tag="bias", name=f"bias_{t}")
                nc.sync.dma_start(out=bias_t[:], in_=bias_d[rows, :kt])
                q_t = sp.tile([P, 1, D], f16, tag="q", name=f"q_{t}")
                nc.sync.dma_start(out=q_t[:, 0, :], in_=query_d[rows, :])

                kemb = kemb_pool.tile([P, kt, D], f16, tag="kemb", name=f"kemb_{t}")
                # split tile 0's kemb gather so products can start sooner
                kck = (kt + 1) // 2 if t == 0 else CK
                for c0 in range(0, kt, kck):
                    ckc = min(kck, kt - c0)
                    ni = P * ckc
                    nc.gpsimd.dma_gather(
                        kemb[:, c0:c0 + ckc, :], ktab_d[:],
                        kidx_t[:, 8 * c0:8 * (c0 + ckc)], ni, ni, D,
                        single_packet=SINGLE_PACKET,
                    )
                stage[t] = (kemb, bias_t, q_t)

            def emit_load_v(t):
                """vemb gathers, deferred — the logits chain (kemb) is the
                critical path; the value phase reads vemb much later."""
                kt = int(k_ts[t])
                off = offs[t]
                vidx_t = sp.tile([P, 8 * kt], i16, tag="vidx", name=f"vidx_{t}")
                nc.sync.dma_start(out=vidx_t[:], in_=vidx_d[:, off:off + 8 * kt])
                vemb = vemb_pool.tile([P, kt, D], f16, tag="vemb", name=f"vemb_{t}")
                for c0 in range(0, kt, CK):
                    ckc = min(CK, kt - c0)
                    ni = P * ckc
                    nc.gpsimd.dma_gather(
                        vemb[:, c0:c0 + ckc, :], vtab_d[:],
                        vidx_t[:, 8 * c0:8 * (c0 + ckc)], ni, ni, D,
                        single_packet=SINGLE_PACKET,
                    )
                stage[("v", t)] = vemb

            def emit_logits(t):
                """logits[p, k] = sum_d q[p, d] * kemb[p, k, d], three-way
                (each engine writes its OWN tile — a shared destination would
                serialize the engines through Tile's WAW tracking):
                k < a_t:       DVE bulk products (2x) + ACT Copy-accum row-sum
                a_t..a_t+b_t:  Pool fused scalar_tensor_tensor (no products)
                rest:          DVE fused scalar_tensor_tensor (1x)
                """
                kt = int(k_ts[t])
                kemb, bias_t, q_t = stage[t]
                a_t = 52 * kt // 100
                b_t = 0

                lg_a = sp.tile([P, max(a_t, 1)], f32, tag="lga", name=f"lga_{t}")
                lg_p = sp.tile([P, max(b_t, 1)], f32, tag="lgp", name=f"lgp_{t}")
                lg_d = sp.tile([P, max(kt - a_t - b_t, 1)], f32, tag="lgd",
                               name=f"lgd_{t}")
                pk = pk_pool.tile([P, max(a_t, 1), D], f16, tag="pk",
                                  name=f"pk_{t}")
                for c0 in range(0, a_t, PCH):
                    cc = min(PCH, a_t - c0)
                    nc.vector.tensor_tensor(
                        out=pk[:, c0:c0 + cc, :],
                        in0=kemb[:, c0:c0 + cc, :],
                        in1=q_t[:].to_broadcast([P, cc, D]),
                        op=mybir.AluOpType.mult,
                    )
                ascr = sp.tile([P, D], f16, tag="ascr", name=f"ascr_{t}")
                for k in range(a_t):
                    nc.scalar.activation(
                        out=ascr[:], in_=pk[:, k, :],
                        func=mybir.ActivationFunctionType.Copy,
                        bias=0.0, scale=1.0,
                        accum_out=lg_a[:, k:k + 1],
                    )
                dscr = sp.tile([P, D], f16, tag="dscr", name=f"dscr_{t}")
                for k in range(a_t + b_t, kt):
                    nc.vector.scalar_tensor_tensor(
                        out=dscr[:], in0=kemb[:, k, :],
                        scalar=1.0, in1=q_t[:, 0, :],
                        op0=mybir.AluOpType.bypass,
                        op1=mybir.AluOpType.mult,
                        accum_out=lg_d[:, k - a_t - b_t:k - a_t - b_t + 1],
                    )
                stage[t] = (kemb, bias_t, q_t, (a_t, b_t),
                            (lg_a, lg_p, lg_d))

            def emit_tail_head(t):
                """bias add (range-wise, folding the three logits shares into
                one tile) -> reduce_max -> Exp(+sumexp) -> 1/sumexp."""
                kt = int(k_ts[t])
                kemb, bias_t, q_t, (a_t, b_t), lgs = stage.pop(t)
                lg_a, lg_p, lg_d = lgs

                logits_t = sp.tile([P, kt], f32, tag="logits", name=f"logits_{t}")
                c = a_t + b_t
                if c < kt:
                    nc.vector.tensor_tensor(
                        out=logits_t[:, c:kt], in0=lg_d[:, 0:kt - c],
                        in1=bias_t[:, c:kt], op=mybir.AluOpType.add)
                if b_t:
                    nc.vector.tensor_tensor(
                        out=logits_t[:, a_t:c], in0=lg_p[:, 0:b_t],
                        in1=bias_t[:, a_t:c], op=mybir.AluOpType.add)
                if a_t:
                    nc.vector.tensor_tensor(
                        out=logits_t[:, 0:a_t], in0=lg_a[:, 0:a_t],
                        in1=bias_t[:, 0:a_t], op=mybir.AluOpType.add)

                neg_max = sp.tile([P, 1], f32, tag="neg_max", name=f"neg_max_{t}")
                nc.vector.tensor_reduce(
                    out=neg_max[:], in_=logits_t[:],
                    axis=mybir.AxisListType.X, op=mybir.AluOpType.max, negate=True,
                )
                probs = sp.tile([P, kt], f32, tag="probs", name=f"probs_{t}")
                sumexp = sp.tile([P, 1], f32, tag="sumexp", name=f"sumexp_{t}")
                nc.scalar.activation(
                    out=probs[:], in_=logits_t[:],
                    func=mybir.ActivationFunctionType.Exp,
                    bias=neg_max[:, :1], scale=1.0,
                    accum_out=sumexp[:],
                )
                inv = sp.tile([P, 1], f32, tag="inv", name=f"inv_{t}")
                nc.vector.reciprocal(out=inv[:], in_=sumexp[:])
                stage[t] = (probs, inv)

            def emit_tail_value(t):
                """knowledge[p, d] = inv * sum_k probs[p, k] * vemb[p, k, d].
                Scaled products split ACT (own tile) / Pool (chained, tail
                tiles only, when gather DGE work has dried up) / DVE, then a
                DVE in-place pairwise tree (fp16 2x) plus two fold-in adds."""
                kt = int(k_ts[t])
                rows = slice(t * P, (t + 1) * P)
                probs, inv = stage.pop(t)
                vemb = stage.pop(("v", t))

                m_a = (35 * kt + 99) // 100
                m_p = 0
                ndve = kt - m_a - m_p

                pact = pact_pool.tile([P, max(m_a, 1), D], f16, tag="pact",
                                      name=f"pact_{t}")
                for k in range(m_a):
                    nc.scalar.activation(
                        out=pact[:, k, :], in_=vemb[:, k, :],
                        func=mybir.ActivationFunctionType.Copy,
                        bias=0.0, scale=probs[:, k:k + 1],
                    )
                pacc = None

                prod = prod_pool.tile([P, max(ndve, 1), D], f16, tag="prod",
                                      name=f"prod_{t}")
                for i, k in enumerate(range(m_a + m_p, kt)):
                    nc.vector.tensor_scalar(
                        out=prod[:, i, :], in0=vemb[:, k, :],
                        scalar1=probs[:, k:k + 1], scalar2=None,
                        op0=mybir.AluOpType.mult,
                    )
                if m_a:
                    nc.vector.tensor_tensor(
                        out=prod[:, 0:m_a, :], in0=prod[:, 0:m_a, :],
                        in1=pact[:, 0:m_a, :], op=mybir.AluOpType.add,
                    )
                n = ndve
                while n > 1:
                    a = n // 2
                    nc.vector.tensor_tensor(
                        out=prod[:, 0:a, :],
                        in0=prod[:, 0:a, :],
                        in1=prod[:, n - a:n, :],
                        op=mybir.AluOpType.add,
                    )
                    n -= a
                if pacc is not None:
                    nc.vector.tensor_tensor(
                        out=prod[:, 0, :], in0=prod[:, 0, :], in1=pacc[:],
                        op=mybir.AluOpType.add,
                    )

                out_t = sp.tile([P, D], f16, tag="out", name=f"out_{t}")
                nc.vector.tensor_scalar(
                    out=out_t[:], in0=prod[:, 0, :], scalar1=inv[:, :1],
                    scalar2=None, op0=mybir.AluOpType.mult,
                )
                nc.sync.dma_start(out=out_d[rows, :], in_=out_t[:])

            # software pipeline: gathers two tiles ahead, logits one ahead,
            # softmax (tail_head) emitted before the next tile's logits so
            # Exp isn't queued behind the next tile's ACT reduces
            emit_load(0)
            emit_load(1)
            emit_logits(0)
            emit_load_v(0)
            for t in range(NTILES):
                if t + 2 < NTILES:
                    emit_load(t + 2)
                emit_tail_head(t)
                if t + 1 < NTILES:
                    emit_logits(t + 1)
                    emit_load_v(t + 1)
                emit_tail_value(t)

    nc.compile()
    return nc


def _get_program(k_ts, nu_k, nu_v):
    key = (tuple(int(k) for k in k_ts), int(nu_k), int(nu_v))
    if key not in _PROGRAM_CACHE:
        _PROGRAM_CACHE[key] = _build_program(*key)
    return _PROGRAM_CACHE[key]


def _wrap_idx(lin):
    """lin[i] (int16) -> [128, NI/16]: idx i at partition i%16, col i//16,
    replicated to the 8 Q7 16-partition groups."""
    ni = lin.size
    a = lin.reshape(ni // 16, 16).T
    return np.ascontiguousarray(np.tile(a, (8, 1)))


def _compact_idx_blocks(ids_sorted, k_ts):
    """ids_sorted: [BC, K] table-row ids (sorted example order). Returns
    (uniq_rows, idx_blocks [128, 8*sum(k_ts)] int16) gathering only the
    j < k_ts[t] prefix columns of each tile."""
    gathered = []
    for t in range(NTILES):
        gathered.append(ids_sorted[t * P:(t + 1) * P, :k_ts[t]])
    flat = np.concatenate([g.ravel() for g in gathered])
    uniq, inv = np.unique(flat, return_inverse=True)
    assert uniq.size < 32768
    inv = inv.astype(np.int16)
    blocks = []
    pos = 0
    for t in range(NTILES):
        kt = k_ts[t]
        tile_inv = inv[pos:pos + P * kt].reshape(P, kt)
        pos += P * kt
        lin = np.ascontiguousarray(tile_inv.T).ravel()  # lin[k*128+p]
        blocks.append(_wrap_idx(lin))
    return uniq, np.concatenate(blocks, axis=1)


def _prepare(keys, values, pair_length, query, key_table, value_table):
    keys = np.asarray(keys).astype(np.int64)
    values = np.asarray(values).astype(np.int64)
    pair_length = np.asarray(pair_length).astype(np.int32)
    query = np.asarray(query, dtype=np.float32)
    ktab16 = np.asarray(key_table, dtype=np.float16)
    vtab16 = np.asarray(value_table, dtype=np.float16)

    # shared per-tile column counts (max over cores so one SPMD program)
    tile_maxes = np.zeros((N_CORES, NTILES), dtype=np.int64)
    perms = []
    for c in range(N_CORES):
        pl = pair_length[c * BC:(c + 1) * BC]
        perm = np.argsort(-pl, kind="stable")
        perms.append(perm)
        pl_s = pl[perm]
        tile_maxes[c] = [pl_s[t * P:(t + 1) * P].max() for t in range(NTILES)]
    k_ts = tuple(int(v) for v in tile_maxes.max(axis=0))

    per_core = []
    inv_perms = []
    nu_k_max = nu_v_max = 0
    for c in range(N_CORES):
        rows = slice(c * BC, (c + 1) * BC)
        perm = perms[c]
        inv = np.empty_like(perm)
        inv[perm] = np.arange(BC)
        inv_perms.append(inv)
        pl_s = pair_length[rows][perm]
        bias = np.where(
            np.arange(K, dtype=np.int32)[None, :] < pl_s[:, None],
            np.float32(0.0), MASK_NEG).astype(np.float32)

        uniq_k, kidx = _compact_idx_blocks(keys[rows][perm], k_ts)
        uniq_v, vidx = _compact_idx_blocks(values[rows][perm], k_ts)
        nu_k_max = max(nu_k_max, uniq_k.size)
        nu_v_max = max(nu_v_max, uniq_v.size)
        per_core.append({
            "kidx": kidx,
            "vidx": vidx,
            "bias": bias,
            "query": np.ascontiguousarray(query[rows][perm]).astype(np.float16),
            "_uniq_k": uniq_k,
            "_uniq_v": uniq_v,
        })

    nu_k = -(-nu_k_max // ROW_PAD) * ROW_PAD
    nu_v = -(-nu_v_max // ROW_PAD) * ROW_PAD
    for c in range(N_CORES):
        m = per_core[c]
        uk, uv = m.pop("_uniq_k"), m.pop("_uniq_v")
        kt_c = np.zeros((nu_k, D), dtype=np.float16)
        kt_c[:uk.size] = ktab16[uk]
        vt_c = np.zeros((nu_v, D), dtype=np.float16)
        vt_c[:uv.size] = vtab16[uv]
        m["key_table"] = kt_c
        m["value_table"] = vt_c
    return per_core, inv_perms, k_ts, nu_k, nu_v


def kernel(keys, values, pair_length, query, key_table, value_table):
    per_core, inv_perms, k_ts, nu_k, nu_v = _prepare(
        keys, values, pair_length, query, key_table, value_table)
    nc = _get_program(k_ts, nu_k, nu_v)
    res = bass_utils.run_bass_kernel_spmd(nc, per_core,
                                          core_ids=list(range(N_CORES)))
    out = np.concatenate(
        [res.results[c]["out"][inv_perms[c]] for c in range(N_CORES)], axis=0)
    return out.astype(np.float32)


# revision 19
# speedup vs baseline: 1.3667x; 1.0117x over previous
"""KVMemory kernel for Trainium2 (8 NeuronCores, Bass/Tile).

Data-parallel over batch: each of the 8 cores handles 512 examples.

Gather strategy: the SWDGE vector-indirect DMA (InstDMACopy) only supports
one index per partition per instruction (~1us descriptor-gen each on Pool —
the old bottleneck at 400 instructions/core). Instead we use the custom
GPSIMD `dma_gather` (InstDMAGatherAnt, mlp library): int16 indices laid out
[128, NI/16] (idx i at partition i%16, col i//16, replicated to all 8
16-partition Q7 groups), gathering up to CK*128 rows per instruction into
[128, CK, 256] fp16 tiles.

int16 only addresses 32767 rows, so the 100K-row tables are COMPACTED per
core on the host: each core references at most 512*50=25600 < 32768 unique
rows; we upload table[unique_rows] per core and remap indices. Tables are
cast to fp16 (tolerance 2e-2; fp16 keeps rel err ~1e-3), halving gather
traffic and enabling the DVE 4x perf mode.

Examples are sorted by pair_length DESCENDING within each core (host perm,
inverted on output). Valid keys are always a prefix (mask = arange(K) <
pair_length), so tile t only gathers/computes K_ts[t] = max pair_length in
that tile columns — ~35% less traffic and DVE work. K_ts and compact-table
sizes are data-dependent; programs are compiled per (k_ts, nu_k, nu_v) and
cached.

Per 128-example tile (engine balance: the fused scalar_tensor_tensor path
runs at 1x on DVE — is_scalar_tensor_tensor disables all DVE perf modes — so
the hot ops are spread across DVE (2x/4x-capable instructions), ACT, and the
Pool/GPSIMD vector path; emission is software-pipelined so tile t+1's
gathers run ahead of tile t's compute):
  - chunked dma_gather of key/value embedding rows (fp16)
  - logits: bulk products kemb*q (tensor_tensor, fp16 2x mode, q broadcast
    on the middle dim keeps the last dim packed) on DVE; per-k row-sums
    split between ACT activation(Copy, accum_out) (pays the 187ns
    read-accumulator tax per op) and one bulk gpsimd.tensor_reduce on Pool
  - masked softmax (host-precomputed -1e30 bias add -> reduce_max negate ->
    ACT Exp with accum row-sum); probs left UNNORMALIZED, 1/sumexp applied
    once to the final [128, 256] accumulator
  - value sum: per-k probs_k*vemb_k split between DVE tensor_scalar (fp16
    4x mode, f32 [128,1] scalar operand exempt) and ACT activation(Copy,
    scale=probs_k), then an in-place pairwise tree reduction over k via
    tensor_tensor adds (fp16 2x) on DVE; the products buffer is shared
    between the logits and value phases of a tile
"""

import sys

if "/opt/trn_rl_repo" not in sys.path:
    sys.path.insert(0, "/opt/trn_rl_repo")

import numpy as np

import concourse.mybir as mybir
import concourse.tile as tile
from concourse import bacc
import concourse.bass_utils as bass_utils

N_CORES = 8
B = 4096
K = 50
D = 256
BC = B // N_CORES          # examples per core
P = 128                    # partition tile (examples per tile)
NTILES = BC // P           # 4
CK = 50                    # max key-columns per dma_gather instruction
PCH = 8                    # key-columns per bulk-products tensor_tensor op
SINGLE_PACKET = False
SCRATCH = 16384            # dynamic_dma_scratch_size (SWDGE ring bytes)
ROW_PAD = 1024             # compact-table row-count quantum (program cache)
MASK_NEG = np.float32(-1e30)

_PROGRAM_CACHE = {}


def _build_program(k_ts, nu_k, nu_v):
    f32 = mybir.dt.float32
    f16 = mybir.dt.float16
    i16 = mybir.dt.int16
    nc = bacc.Bacc("TRN2", target_bir_lowering=False, debug=False,
                   num_devices=N_CORES, dynamic_dma_scratch_size=SCRATCH)

    sum_k = sum(k_ts)
    kidx_d = nc.dram_tensor("kidx", [P, 8 * sum_k], i16, kind="ExternalInput")
    vidx_d = nc.dram_tensor("vidx", [P, 8 * sum_k], i16, kind="ExternalInput")
    bias_d = nc.dram_tensor("bias", [BC, K], f32, kind="ExternalInput")
    query_d = nc.dram_tensor("query", [BC, D], f16, kind="ExternalInput")
    ktab_d = nc.dram_tensor("key_table", [nu_k, D], f16, kind="ExternalInput")
    vtab_d = nc.dram_tensor("value_table", [nu_v, D], f16, kind="ExternalInput")
    out_d = nc.dram_tensor("out", [BC, D], f16, kind="ExternalOutput")

    with tile.TileContext(nc) as tc:
        with (
            tc.tile_pool(name="kemb", bufs=2) as kemb_pool,
            tc.tile_pool(name="vemb", bufs=3) as vemb_pool,
            tc.tile_pool(name="pk", bufs=2) as pk_pool,
            tc.tile_pool(name="prod", bufs=2) as prod_pool,
            tc.tile_pool(name="pact", bufs=1) as pact_pool,
            tc.tile_pool(name="small", bufs=3) as sp,
        ):
            offs = [8 * sum(int(k) for k in k_ts[:t]) for t in range(NTILES)]
            stage = {}

            def emit_load(t):
                kt = int(k_ts[t])
                rows = slice(t * P, (t + 1) * P)
                off = offs[t]

                kidx_t = sp.tile([P, 8 * kt], i16, tag="kidx", name=f"kidx_{t}")
                nc.sync.dma_start(out=kidx_t[:], in_=kidx_d[:, off:off + 8 * kt])
                bias_t = sp.tile([P, kt], f32, tag="bias", name=f"bias_{t}")
                nc.sync.dma_start(out=bias_t[:], in_=bias_d[rows, :kt])
                q_t = sp.tile([P, 1, D], f16, tag="q", name=f"q_{t}")
                nc.sync.dma_start(out=q_t[:, 0, :], in_=query_d[rows, :])

                kemb = kemb_pool.tile([P, kt, D], f16, tag="kemb", name=f"kemb_{t}")
                # split early tiles' kemb gathers so products start sooner
                # (pipeline fill); Pool DGE has plenty of headroom
                kck = {0: (kt + 3) // 4, 1: (kt + 1) // 2}.get(t, CK)
                for c0 in range(0, kt, kck):
                    ckc = min(kck, kt - c0)
                    ni = P * ckc
                    nc.gpsimd.dma_gather(
                        kemb[:, c0:c0 + ckc, :], ktab_d[:],
                        kidx_t[:, 8 * c0:8 * (c0 + ckc)], ni, ni, D,
                        single_packet=SINGLE_PACKET,
                    )
                stage[t] = (kemb, bias_t, q_t)

            def emit_load_v(t):
                """vemb gathers, deferred — the logits chain (kemb) is the
                critical path; the value phase reads vemb much later."""
                kt = int(k_ts[t])
                off = offs[t]
                vidx_t = sp.tile([P, 8 * kt], i16, tag="vidx", name=f"vidx_{t}")
                nc.sync.dma_start(out=vidx_t[:], in_=vidx_d[:, off:off + 8 * kt])
                vemb = vemb_pool.tile([P, kt, D], f16, tag="vemb", name=f"vemb_{t}")
                for c0 in range(0, kt, CK):
                    ckc = min(CK, kt - c0)
                    ni = P * ckc
                    nc.gpsimd.dma_gather(
                        vemb[:, c0:c0 + ckc, :], vtab_d[:],
                        vidx_t[:, 8 * c0:8 * (c0 + ckc)], ni, ni, D,
                        single_packet=SINGLE_PACKET,
                    )
                stage[("v", t)] = vemb

            def emit_logits(t):
                """logits[p, k] = sum_d q[p, d] * kemb[p, k, d], three-way
                (each engine writes its OWN tile — a shared destination would
                serialize the engines through Tile's WAW tracking):
                k < a_t:       DVE bulk products (2x) + ACT Copy-accum row-sum
                a_t..a_t+b_t:  Pool fused scalar_tensor_tensor (no products)
                rest:          DVE fused scalar_tensor_tensor (1x)
                """
                kt = int(k_ts[t])
                kemb, bias_t, q_t = stage[t]
                a_t = 52 * kt // 100
                b_t = 0

                lg_a = sp.tile([P, max(a_t, 1)], f32, tag="lga", name=f"lga_{t}")
                lg_p = sp.tile([P, max(b_t, 1)], f32, tag="lgp", name=f"lgp_{t}")
                lg_d = sp.tile([P, max(kt - a_t - b_t, 1)], f32, tag="lgd",
                               name=f"lgd_{t}")
                pk = pk_pool.tile([P, max(a_t, 1), D], f16, tag="pk",
                                  name=f"pk_{t}")
                for c0 in range(0, a_t, PCH):
                    cc = min(PCH, a_t - c0)
                    nc.vector.tensor_tensor(
                        out=pk[:, c0:c0 + cc, :],
                        in0=kemb[:, c0:c0 + cc, :],
                        in1=q_t[:].to_broadcast([P, cc, D]),
                        op=mybir.AluOpType.mult,
                    )
                ascr = sp.tile([P, D], f16, tag="ascr", name=f"ascr_{t}")
                for k in range(a_t):
                    nc.scalar.activation(
                        out=ascr[:], in_=pk[:, k, :],
                        func=mybir.ActivationFunctionType.Copy,
                        bias=0.0, scale=1.0,
                        accum_out=lg_a[:, k:k + 1],
                    )
                dscr = sp.tile([P, D], f16, tag="dscr", name=f"dscr_{t}")
                for k in range(a_t + b_t, kt):
                    nc.vector.scalar_tensor_tensor(
                        out=dscr[:], in0=kemb[:, k, :],
                        scalar=1.0, in1=q_t[:, 0, :],
                        op0=mybir.AluOpType.bypass,
                        op1=mybir.AluOpType.mult,
                        accum_out=lg_d[:, k - a_t - b_t:k - a_t - b_t + 1],
                    )
                stage[t] = (kemb, bias_t, q_t, (a_t, b_t),
                            (lg_a, lg_p, lg_d))

            def emit_tail_head(t):
                """bias add (range-wise, folding the three logits shares into
                one tile) -> reduce_max -> Exp(+sumexp) -> 1/sumexp."""
                kt = int(k_ts[t])
                kemb, bias_t, q_t, (a_t, b_t), lgs = stage.pop(t)
                lg_a, lg_p, lg_d = lgs

                logits_t = sp.tile([P, kt], f32, tag="logits", name=f"logits_{t}")
                c = a_t + b_t
                if c < kt:
                    nc.vector.tensor_tensor(
                        out=logits_t[:, c:kt], in0=lg_d[:, 0:kt - c],
                        in1=bias_t[:, c:kt], op=mybir.AluOpType.add)
                if b_t:
                    nc.vector.tensor_tensor(
                        out=logits_t[:, a_t:c], in0=lg_p[:, 0:b_t],
                        in1=bias_t[:, a_t:c], op=mybir.AluOpType.add)
                if a_t:
                    nc.vector.tensor_tensor(
                        out=logits_t[:, 0:a_t], in0=lg_a[:, 0:a_t],
                        in1=bias_t[:, 0:a_t], op=mybir.AluOpType.add)

                neg_max = sp.tile([P, 1], f32, tag="neg_max", name=f"neg_max_{t}")
                nc.vector.tensor_reduce(
                    out=neg_max[:], in_=logits_t[:],
                    axis=mybir.AxisListType.X, op=mybir.AluOpType.max, negate=True,
                )
                probs = sp.tile([P, kt], f32, tag="probs", name=f"probs_{t}")
                sumexp = sp.tile([P, 1], f32, tag="sumexp", name=f"sumexp_{t}")
                nc.scalar.activation(
                    out=probs[:], in_=logits_t[:],
                    func=mybir.ActivationFunctionType.Exp,
                    bias=neg_max[:, :1], scale=1.0,
                    accum_out=sumexp[:],
                )
                inv = sp.tile([P, 1], f32, tag="inv", name=f"inv_{t}")
                nc.vector.reciprocal(out=inv[:], in_=sumexp[:])
                stage[t] = (probs, inv)

            def emit_tail_value(t):
                """knowledge[p, d] = inv * sum_k probs[p, k] * vemb[p, k, d].
                Scaled products split ACT (own tile) / Pool (chained, tail
                tiles only, when gather DGE work has dried up) / DVE, then a
                DVE in-place pairwise tree (fp16 2x) plus two fold-in adds."""
                kt = int(k_ts[t])
                rows = slice(t * P, (t + 1) * P)
                probs, inv = stage.pop(t)
                vemb = stage.pop(("v", t))

                m_a = (35 * kt + 99) // 100
                m_p = 0
                ndve = kt - m_a - m_p

                pact = pact_pool.tile([P, max(m_a, 1), D], f16, tag="pact",
                                      name=f"pact_{t}")
                for k in range(m_a):
                    nc.scalar.activation(
                        out=pact[:, k, :], in_=vemb[:, k, :],
                        func=mybir.ActivationFunctionType.Copy,
                        bias=0.0, scale=probs[:, k:k + 1],
                    )
                pacc = None

                prod = prod_pool.tile([P, max(ndve, 1), D], f16, tag="prod",
                                      name=f"prod_{t}")
                for i, k in enumerate(range(m_a + m_p, kt)):
                    nc.vector.tensor_scalar(
                        out=prod[:, i, :], in0=vemb[:, k, :],
                        scalar1=probs[:, k:k + 1], scalar2=None,
                        op0=mybir.AluOpType.mult,
                    )
                if m_a:
                    nc.vector.tensor_tensor(
                        out=prod[:, 0:m_a, :], in0=prod[:, 0:m_a, :],
                        in1=pact[:, 0:m_a, :], op=mybir.AluOpType.add,
                    )
                n = ndve
                while n > 1:
                    a = n // 2
                    nc.vector.tensor_tensor(
                        out=prod[:, 0:a, :],
                        in0=prod[:, 0:a, :],
                        in1=prod[:, n - a:n, :],
                        op=mybir.AluOpType.add,
                    )
                    n -= a
                if pacc is not None:
                    nc.vector.tensor_tensor(
                        out=prod[:, 0, :], in0=prod[:, 0, :], in1=pacc[:],
                        op=mybir.AluOpType.add,
                    )

                out_t = sp.tile([P, D], f16, tag="out", name=f"out_{t}")
                nc.vector.tensor_scalar(
                    out=out_t[:], in0=prod[:, 0, :], scalar1=inv[:, :1],
                    scalar2=None, op0=mybir.AluOpType.mult,
                )
                nc.sync.dma_start(out=out_d[rows, :], in_=out_t[:])

            # software pipeline: gathers two tiles ahead, logits one ahead,
            # softmax (tail_head) emitted before the next tile's logits so
            # Exp isn't queued behind the next tile's ACT reduces
            emit_load(0)
            emit_load(1)
            emit_logits(0)
            emit_load_v(0)
            for t in range(NTILES):
                if t + 2 < NTILES:
                    emit_load(t + 2)
                emit_tail_head(t)
                if t + 1 < NTILES:
                    emit_logits(t + 1)
                    emit_load_v(t + 1)
                emit_tail_value(t)

    nc.compile()
    return nc


def _get_program(k_ts, nu_k, nu_v):
    key = (tuple(int(k) for k in k_ts), int(nu_k), int(nu_v))
    if key not in _PROGRAM_CACHE:
        _PROGRAM_CACHE[key] = _build_program(*key)
    return _PROGRAM_CACHE[key]


def _wrap_idx(lin):
    """lin[i] (int16) -> [128, NI/16]: idx i at partition i%16, col i//16,
    replicated to the 8 Q7 16-partition groups."""
    ni = lin.size
    a = lin.reshape(ni // 16, 16).T
    return np.ascontiguousarray(np.tile(a, (8, 1)))


def _compact_idx_blocks(ids_sorted, k_ts):
    """ids_sorted: [BC, K] table-row ids (sorted example order). Returns
    (uniq_rows, idx_blocks [128, 8*sum(k_ts)] int16) gathering only the
    j < k_ts[t] prefix columns of each tile."""
    gathered = []
    for t in range(NTILES):
        gathered.append(ids_sorted[t * P:(t + 1) * P, :k_ts[t]])
    flat = np.concatenate([g.ravel() for g in gathered])
    uniq, inv = np.unique(flat, return_inverse=True)
    assert uniq.size < 32768
    inv = inv.astype(np.int16)
    blocks = []
    pos = 0
    for t in range(NTILES):
        kt = k_ts[t]
        tile_inv = inv[pos:pos + P * kt].reshape(P, kt)
        pos += P * kt
        lin = np.ascontiguousarray(tile_inv.T).ravel()  # lin[k*128+p]
        blocks.append(_wrap_idx(lin))
    return uniq, np.concatenate(blocks, axis=1)


def _prepare(keys, values, pair_length, query, key_table, value_table):
    keys = np.asarray(keys).astype(np.int64)
    values = np.asarray(values).astype(np.int64)
    pair_length = np.asarray(pair_length).astype(np.int32)
    query = np.asarray(query, dtype=np.float32)
    ktab16 = np.asarray(key_table, dtype=np.float16)
    vtab16 = np.asarray(value_table, dtype=np.float16)

    # shared per-tile column counts (max over cores so one SPMD program)
    tile_maxes = np.zeros((N_CORES, NTILES), dtype=np.int64)
    perms = []
    for c in range(N_CORES):
        pl = pair_length[c * BC:(c + 1) * BC]
        perm = np.argsort(-pl, kind="stable")
        perms.append(perm)
        pl_s = pl[perm]
        tile_maxes[c] = [pl_s[t * P:(t + 1) * P].max() for t in range(NTILES)]
    k_ts = tuple(int(v) for v in tile_maxes.max(axis=0))

    per_core = []
    inv_perms = []
    nu_k_max = nu_v_max = 0
    for c in range(N_CORES):
        rows = slice(c * BC, (c + 1) * BC)
        perm = perms[c]
        inv = np.empty_like(perm)
        inv[perm] = np.arange(BC)
        inv_perms.append(inv)
        pl_s = pair_length[rows][perm]
        bias = np.where(
            np.arange(K, dtype=np.int32)[None, :] < pl_s[:, None],
            np.float32(0.0), MASK_NEG).astype(np.float32)

        uniq_k, kidx = _compact_idx_blocks(keys[rows][perm], k_ts)
        uniq_v, vidx = _compact_idx_blocks(values[rows][perm], k_ts)
        nu_k_max = max(nu_k_max, uniq_k.size)
        nu_v_max = max(nu_v_max, uniq_v.size)
        per_core.append({
            "kidx": kidx,
            "vidx": vidx,
            "bias": bias,
            "query": np.ascontiguousarray(query[rows][perm]).astype(np.float16),
            "_uniq_k": uniq_k,
            "_uniq_v": uniq_v,
        })

    nu_k = -(-nu_k_max // ROW_PAD) * ROW_PAD
    nu_v = -(-nu_v_max // ROW_PAD) * ROW_PAD
    for c in range(N_CORES):
        m = per_core[c]
        uk, uv = m.pop("_uniq_k"), m.pop("_uniq_v")
        kt_c = np.zeros((nu_k, D), dtype=np.float16)
        kt_c[:uk.size] = ktab16[uk]
        vt_c = np.zeros((nu_v, D), dtype=np.float16)
        vt_c[:uv.size] = vtab16[uv]
        m["key_table"] = kt_c
        m["value_table"] = vt_c
    return per_core, inv_perms, k_ts, nu_k, nu_v


def kernel(keys, values, pair_length, query, key_table, value_table):
    per_core, inv_perms, k_ts, nu_k, nu_v = _prepare(
        keys, values, pair_length, query, key_table, value_table)
    nc = _get_program(k_ts, nu_k, nu_v)
    res = bass_utils.run_bass_kernel_spmd(nc, per_core,
                                          core_ids=list(range(N_CORES)))
    out = np.concatenate(
        [res.results[c]["out"][inv_perms[c]] for c in range(N_CORES)], axis=0)
    return out.astype(np.float32)


# revision 20
# speedup vs baseline: 1.3879x; 1.0155x over previous
"""KVMemory kernel for Trainium2 (8 NeuronCores, Bass/Tile).

Data-parallel over batch: each of the 8 cores handles 512 examples.

Gather strategy: the SWDGE vector-indirect DMA (InstDMACopy) only supports
one index per partition per instruction (~1us descriptor-gen each on Pool —
the old bottleneck at 400 instructions/core). Instead we use the custom
GPSIMD `dma_gather` (InstDMAGatherAnt, mlp library): int16 indices laid out
[128, NI/16] (idx i at partition i%16, col i//16, replicated to all 8
16-partition Q7 groups), gathering up to CK*128 rows per instruction into
[128, CK, 256] fp16 tiles.

int16 only addresses 32767 rows, so the 100K-row tables are COMPACTED per
core on the host: each core references at most 512*50=25600 < 32768 unique
rows; we upload table[unique_rows] per core and remap indices. Tables are
cast to fp16 (tolerance 2e-2; fp16 keeps rel err ~1e-3), halving gather
traffic and enabling the DVE 4x perf mode.

Examples are sorted by pair_length DESCENDING within each core (host perm,
inverted on output). Valid keys are always a prefix (mask = arange(K) <
pair_length), so tile t only gathers/computes K_ts[t] = max pair_length in
that tile columns — ~35% less traffic and DVE work. K_ts and compact-table
sizes are data-dependent; programs are compiled per (k_ts, nu_k, nu_v) and
cached.

Per 128-example tile (engine balance: the fused scalar_tensor_tensor path
runs at 1x on DVE — is_scalar_tensor_tensor disables all DVE perf modes — so
the hot ops are spread across DVE (2x/4x-capable instructions), ACT, and the
Pool/GPSIMD vector path; emission is software-pipelined so tile t+1's
gathers run ahead of tile t's compute):
  - chunked dma_gather of key/value embedding rows (fp16)
  - logits: bulk products kemb*q (tensor_tensor, fp16 2x mode, q broadcast
    on the middle dim keeps the last dim packed) on DVE; per-k row-sums
    split between ACT activation(Copy, accum_out) (pays the 187ns
    read-accumulator tax per op) and one bulk gpsimd.tensor_reduce on Pool
  - masked softmax (host-precomputed -1e30 bias add -> reduce_max negate ->
    ACT Exp with accum row-sum); probs left UNNORMALIZED, 1/sumexp applied
    once to the final [128, 256] accumulator
  - value sum: per-k probs_k*vemb_k split between DVE tensor_scalar (fp16
    4x mode, f32 [128,1] scalar operand exempt) and ACT activation(Copy,
    scale=probs_k), then an in-place pairwise tree reduction over k via
    tensor_tensor adds (fp16 2x) on DVE; the products buffer is shared
    between the logits and value phases of a tile
"""

import sys

if "/opt/trn_rl_repo" not in sys.path:
    sys.path.insert(0, "/opt/trn_rl_repo")

import numpy as np

import concourse.mybir as mybir
import concourse.tile as tile
from concourse import bacc
import concourse.bass_utils as bass_utils

N_CORES = 8
B = 4096
K = 50
D = 256
BC = B // N_CORES          # examples per core
P = 128                    # partition tile (examples per tile)
NTILES = BC // P           # 4
CK = 50                    # max key-columns per dma_gather instruction
PCH = 8                    # key-columns per bulk-products tensor_tensor op
SINGLE_PACKET = False
SCRATCH = 16384            # dynamic_dma_scratch_size (SWDGE ring bytes)
ROW_PAD = 1024             # compact-table row-count quantum (program cache)
MASK_NEG = np.float32(-1e30)

_PROGRAM_CACHE = {}


def _build_program(k_ts, nu_k, nu_v):
    f32 = mybir.dt.float32
    f16 = mybir.dt.float16
    i16 = mybir.dt.int16
    nc = bacc.Bacc("TRN2", target_bir_lowering=False, debug=False,
                   num_devices=N_CORES, dynamic_dma_scratch_size=SCRATCH)

    sum_k = sum(k_ts)
    kidx_d = nc.dram_tensor("kidx", [P, 8 * sum_k], i16, kind="ExternalInput")
    vidx_d = nc.dram_tensor("vidx", [P, 8 * sum_k], i16, kind="ExternalInput")
    bias_d = nc.dram_tensor("bias", [BC, K], f32, kind="ExternalInput")
    query_d = nc.dram_tensor("query", [BC, D], f16, kind="ExternalInput")
    ktab_d = nc.dram_tensor("key_table", [nu_k, D], f16, kind="ExternalInput")
    vtab_d = nc.dram_tensor("value_table", [nu_v, D], f16, kind="ExternalInput")
    out_d = nc.dram_tensor("out", [BC, D], f16, kind="ExternalOutput")

    with tile.TileContext(nc) as tc:
        with (
            tc.tile_pool(name="kemb", bufs=2) as kemb_pool,
            tc.tile_pool(name="vemb", bufs=2) as vemb_pool,
            tc.tile_pool(name="pk", bufs=2) as pk_pool,
            tc.tile_pool(name="prod", bufs=2) as prod_pool,
            tc.tile_pool(name="pact", bufs=1) as pact_pool,
            tc.tile_pool(name="ptree", bufs=2) as ptree_pool,
            tc.tile_pool(name="small", bufs=3) as sp,
        ):
            offs = [8 * sum(int(k) for k in k_ts[:t]) for t in range(NTILES)]
            stage = {}

            def emit_load(t):
                kt = int(k_ts[t])
                rows = slice(t * P, (t + 1) * P)
                off = offs[t]

                kidx_t = sp.tile([P, 8 * kt], i16, tag="kidx", name=f"kidx_{t}")
                nc.sync.dma_start(out=kidx_t[:], in_=kidx_d[:, off:off + 8 * kt])
                bias_t = sp.tile([P, kt], f32, tag="bias", name=f"bias_{t}")
                nc.sync.dma_start(out=bias_t[:], in_=bias_d[rows, :kt])
                q_t = sp.tile([P, 1, D], f16, tag="q", name=f"q_{t}")
                nc.sync.dma_start(out=q_t[:, 0, :], in_=query_d[rows, :])

                kemb = kemb_pool.tile([P, kt, D], f16, tag="kemb", name=f"kemb_{t}")
                # split early tiles' kemb gathers so products start sooner
                # (pipeline fill); Pool DGE has plenty of headroom
                kck = {0: (kt + 3) // 4, 1: (kt + 1) // 2}.get(t, CK)
                for c0 in range(0, kt, kck):
                    ckc = min(kck, kt - c0)
                    ni = P * ckc
                    nc.gpsimd.dma_gather(
                        kemb[:, c0:c0 + ckc, :], ktab_d[:],
                        kidx_t[:, 8 * c0:8 * (c0 + ckc)], ni, ni, D,
                        single_packet=SINGLE_PACKET,
                    )
                stage[t] = (kemb, bias_t, q_t)

            def emit_load_v(t):
                """vemb gathers, deferred — the logits chain (kemb) is the
                critical path; the value phase reads vemb much later."""
                kt = int(k_ts[t])
                off = offs[t]
                vidx_t = sp.tile([P, 8 * kt], i16, tag="vidx", name=f"vidx_{t}")
                nc.sync.dma_start(out=vidx_t[:], in_=vidx_d[:, off:off + 8 * kt])
                vemb = vemb_pool.tile([P, kt, D], f16, tag="vemb", name=f"vemb_{t}")
                for c0 in range(0, kt, CK):
                    ckc = min(CK, kt - c0)
                    ni = P * ckc
                    nc.gpsimd.dma_gather(
                        vemb[:, c0:c0 + ckc, :], vtab_d[:],
                        vidx_t[:, 8 * c0:8 * (c0 + ckc)], ni, ni, D,
                        single_packet=SINGLE_PACKET,
                    )
                stage[("v", t)] = vemb

            def emit_logits(t):
                """logits[p, k] = sum_d q[p, d] * kemb[p, k, d], three-way
                (each engine writes its OWN tile — a shared destination would
                serialize the engines through Tile's WAW tracking):
                k < a_t:       DVE bulk products (2x) + ACT Copy-accum row-sum
                a_t..a_t+b_t:  Pool fused scalar_tensor_tensor (no products)
                rest:          DVE fused scalar_tensor_tensor (1x)
                """
                kt = int(k_ts[t])
                kemb, bias_t, q_t = stage[t]
                a_t = 45 * kt // 100
                b_t = 0

                lg_a = sp.tile([P, max(a_t, 1)], f32, tag="lga", name=f"lga_{t}")
                lg_p = sp.tile([P, max(b_t, 1)], f32, tag="lgp", name=f"lgp_{t}")
                lg_d = sp.tile([P, max(kt - a_t - b_t, 1)], f32, tag="lgd",
                               name=f"lgd_{t}")
                pk = pk_pool.tile([P, max(a_t, 1), D], f16, tag="pk",
                                  name=f"pk_{t}")
                for c0 in range(0, a_t, PCH):
                    cc = min(PCH, a_t - c0)
                    nc.vector.tensor_tensor(
                        out=pk[:, c0:c0 + cc, :],
                        in0=kemb[:, c0:c0 + cc, :],
                        in1=q_t[:].to_broadcast([P, cc, D]),
                        op=mybir.AluOpType.mult,
                    )
                ascr = sp.tile([P, D], f16, tag="ascr", name=f"ascr_{t}")
                for k in range(a_t):
                    nc.scalar.activation(
                        out=ascr[:], in_=pk[:, k, :],
                        func=mybir.ActivationFunctionType.Copy,
                        bias=0.0, scale=1.0,
                        accum_out=lg_a[:, k:k + 1],
                    )
                # DVE share: bulk 2x products into ptree, then an
                # in-place pairwise fold over d — logits land (fp16,
                # stride-D) at ptree[:, :, 0]
                nk = kt - a_t - b_t
                ptree = ptree_pool.tile([P, max(nk, 1), D], f16, tag="ptree",
                                        name=f"ptree_{t}")
                for c0 in range(0, nk, PCH):
                    cc = min(PCH, nk - c0)
                    nc.vector.tensor_tensor(
                        out=ptree[:, c0:c0 + cc, :],
                        in0=kemb[:, a_t + b_t + c0:a_t + b_t + c0 + cc, :],
                        in1=q_t[:].to_broadcast([P, cc, D]),
                        op=mybir.AluOpType.mult,
                    )
                w = D
                while w > 1:
                    h = w // 2
                    nc.vector.tensor_tensor(
                        out=ptree[:, 0:nk, 0:h],
                        in0=ptree[:, 0:nk, 0:h],
                        in1=ptree[:, 0:nk, w - h:w],
                        op=mybir.AluOpType.add,
                    )
                    w -= h
                stage[t] = (kemb, bias_t, q_t, (a_t, b_t),
                            (lg_a, lg_p, ptree))

            def emit_tail_head(t):
                """bias add (range-wise, folding the three logits shares into
                one tile) -> reduce_max -> Exp(+sumexp) -> 1/sumexp."""
                kt = int(k_ts[t])
                kemb, bias_t, q_t, (a_t, b_t), lgs = stage.pop(t)
                lg_a, lg_p, lg_d = lgs

                logits_t = sp.tile([P, kt], f32, tag="logits", name=f"logits_{t}")
                c = a_t + b_t
                if c < kt:
                    nc.vector.tensor_tensor(
                        out=logits_t[:, c:kt], in0=lg_d[:, 0:kt - c, 0],
                        in1=bias_t[:, c:kt], op=mybir.AluOpType.add)
                if b_t:
                    nc.vector.tensor_tensor(
                        out=logits_t[:, a_t:c], in0=lg_p[:, 0:b_t],
                        in1=bias_t[:, a_t:c], op=mybir.AluOpType.add)
                if a_t:
                    nc.vector.tensor_tensor(
                        out=logits_t[:, 0:a_t], in0=lg_a[:, 0:a_t],
                        in1=bias_t[:, 0:a_t], op=mybir.AluOpType.add)

                neg_max = sp.tile([P, 1], f32, tag="neg_max", name=f"neg_max_{t}")
                nc.vector.tensor_reduce(
                    out=neg_max[:], in_=logits_t[:],
                    axis=mybir.AxisListType.X, op=mybir.AluOpType.max, negate=True,
                )
                probs = sp.tile([P, kt], f32, tag="probs", name=f"probs_{t}")
                sumexp = sp.tile([P, 1], f32, tag="sumexp", name=f"sumexp_{t}")
                nc.scalar.activation(
                    out=probs[:], in_=logits_t[:],
                    func=mybir.ActivationFunctionType.Exp,
                    bias=neg_max[:, :1], scale=1.0,
                    accum_out=sumexp[:],
                )
                inv = sp.tile([P, 1], f32, tag="inv", name=f"inv_{t}")
                nc.vector.reciprocal(out=inv[:], in_=sumexp[:])
                stage[t] = (probs, inv)

            def emit_tail_value(t):
                """knowledge[p, d] = inv * sum_k probs[p, k] * vemb[p, k, d].
                Scaled products split ACT (own tile) / Pool (chained, tail
                tiles only, when gather DGE work has dried up) / DVE, then a
                DVE in-place pairwise tree (fp16 2x) plus two fold-in adds."""
                kt = int(k_ts[t])
                rows = slice(t * P, (t + 1) * P)
                probs, inv = stage.pop(t)
                vemb = stage.pop(("v", t))

                m_a = (40 * kt + 99) // 100
                m_p = 0
                ndve = kt - m_a - m_p

                pact = pact_pool.tile([P, max(m_a, 1), D], f16, tag="pact",
                                      name=f"pact_{t}")
                for k in range(m_a):
                    nc.scalar.activation(
                        out=pact[:, k, :], in_=vemb[:, k, :],
                        func=mybir.ActivationFunctionType.Copy,
                        bias=0.0, scale=probs[:, k:k + 1],
                    )
                pacc = None

                prod = prod_pool.tile([P, max(ndve, 1), D], f16, tag="prod",
                                      name=f"prod_{t}")
                for i, k in enumerate(range(m_a + m_p, kt)):
                    nc.vector.tensor_scalar(
                        out=prod[:, i, :], in0=vemb[:, k, :],
                        scalar1=probs[:, k:k + 1], scalar2=None,
                        op0=mybir.AluOpType.mult,
                    )
                if m_a:
                    nc.vector.tensor_tensor(
                        out=prod[:, 0:m_a, :], in0=prod[:, 0:m_a, :],
                        in1=pact[:, 0:m_a, :], op=mybir.AluOpType.add,
                    )
                n = ndve
                while n > 1:
                    a = n // 2
                    nc.vector.tensor_tensor(
                        out=prod[:, 0:a, :],
                        in0=prod[:, 0:a, :],
                        in1=prod[:, n - a:n, :],
                        op=mybir.AluOpType.add,
                    )
                    n -= a
                if pacc is not None:
                    nc.vector.tensor_tensor(
                        out=prod[:, 0, :], in0=prod[:, 0, :], in1=pacc[:],
                        op=mybir.AluOpType.add,
                    )

                out_t = sp.tile([P, D], f16, tag="out", name=f"out_{t}")
                nc.vector.tensor_scalar(
                    out=out_t[:], in0=prod[:, 0, :], scalar1=inv[:, :1],
                    scalar2=None, op0=mybir.AluOpType.mult,
                )
                nc.sync.dma_start(out=out_d[rows, :], in_=out_t[:])

            # software pipeline: gathers two tiles ahead, logits one ahead,
            # softmax (tail_head) emitted before the next tile's logits so
            # Exp isn't queued behind the next tile's ACT reduces
            emit_load(0)
            emit_load(1)
            emit_logits(0)
            emit_load_v(0)
            for t in range(NTILES):
                if t + 2 < NTILES:
                    emit_load(t + 2)
                emit_tail_head(t)
                if t + 1 < NTILES:
                    emit_logits(t + 1)
                    emit_load_v(t + 1)
                emit_tail_value(t)

    nc.compile()
    return nc


def _get_program(k_ts, nu_k, nu_v):
    key = (tuple(int(k) for k in k_ts), int(nu_k), int(nu_v))
    if key not in _PROGRAM_CACHE:
        _PROGRAM_CACHE[key] = _build_program(*key)
    return _PROGRAM_CACHE[key]


def _wrap_idx(lin):
    """lin[i] (int16) -> [128, NI/16]: idx i at partition i%16, col i//16,
    replicated to the 8 Q7 16-partition groups."""
    ni = lin.size
    a = lin.reshape(ni // 16, 16).T
    return np.ascontiguousarray(np.tile(a, (8, 1)))


def _compact_idx_blocks(ids_sorted, k_ts):
    """ids_sorted: [BC, K] table-row ids (sorted example order). Returns
    (uniq_rows, idx_blocks [128, 8*sum(k_ts)] int16) gathering only the
    j < k_ts[t] prefix columns of each tile."""
    gathered = []
    for t in range(NTILES):
        gathered.append(ids_sorted[t * P:(t + 1) * P, :k_ts[t]])
    flat = np.concatenate([g.ravel() for g in gathered])
    uniq, inv = np.unique(flat, return_inverse=True)
    assert uniq.size < 32768
    inv = inv.astype(np.int16)
    blocks = []
    pos = 0
    for t in range(NTILES):
        kt = k_ts[t]
        tile_inv = inv[pos:pos + P * kt].reshape(P, kt)
        pos += P * kt
        lin = np.ascontiguousarray(tile_inv.T).ravel()  # lin[k*128+p]
        blocks.append(_wrap_idx(lin))
    return uniq, np.concatenate(blocks, axis=1)


def _prepare(keys, values, pair_length, query, key_table, value_table):
    keys = np.asarray(keys).astype(np.int64)
    values = np.asarray(values).astype(np.int64)
    pair_length = np.asarray(pair_length).astype(np.int32)
    query = np.asarray(query, dtype=np.float32)
    ktab16 = np.asarray(key_table, dtype=np.float16)
    vtab16 = np.asarray(value_table, dtype=np.float16)

    # shared per-tile column counts (max over cores so one SPMD program)
    tile_maxes = np.zeros((N_CORES, NTILES), dtype=np.int64)
    perms = []
    for c in range(N_CORES):
        pl = pair_length[c * BC:(c + 1) * BC]
        perm = np.argsort(-pl, kind="stable")
        perms.append(perm)
        pl_s = pl[perm]
        tile_maxes[c] = [pl_s[t * P:(t + 1) * P].max() for t in range(NTILES)]
    k_ts = tuple(int(v) for v in tile_maxes.max(axis=0))

    per_core = []
    inv_perms = []
    nu_k_max = nu_v_max = 0
    for c in range(N_CORES):
        rows = slice(c * BC, (c + 1) * BC)
        perm = perms[c]
        inv = np.empty_like(perm)
        inv[perm] = np.arange(BC)
        inv_perms.append(inv)
        pl_s = pair_length[rows][perm]
        bias = np.where(
            np.arange(K, dtype=np.int32)[None, :] < pl_s[:, None],
            np.float32(0.0), MASK_NEG).astype(np.float32)

        uniq_k, kidx = _compact_idx_blocks(keys[rows][perm], k_ts)
        uniq_v, vidx = _compact_idx_blocks(values[rows][perm], k_ts)
        nu_k_max = max(nu_k_max, uniq_k.size)
        nu_v_max = max(nu_v_max, uniq_v.size)
        per_core.append({
            "kidx": kidx,
            "vidx": vidx,
            "bias": bias,
            "query": np.ascontiguousarray(query[rows][perm]).astype(np.float16),
            "_uniq_k": uniq_k,
            "_uniq_v": uniq_v,
        })

    nu_k = -(-nu_k_max // ROW_PAD) * ROW_PAD
    nu_v = -(-nu_v_max // ROW_PAD) * ROW_PAD
    for c in range(N_CORES):
        m = per_core[c]
        uk, uv = m.pop("_uniq_k"), m.pop("_uniq_v")
        kt_c = np.zeros((nu_k, D), dtype=np.float16)
        kt_c[:uk.size] = ktab16[uk]
        vt_c = np.zeros((nu_v, D), dtype=np.float16)
        vt_c[:uv.size] = vtab16[uv]
        m["key_table"] = kt_c
        m["value_table"] = vt_c
    return per_core, inv_perms, k_ts, nu_k, nu_v


def kernel(keys, values, pair_length, query, key_table, value_table):
    per_core, inv_perms, k_ts, nu_k, nu_v = _prepare(
        keys, values, pair_length, query, key_table, value_table)
    nc = _get_program(k_ts, nu_k, nu_v)
    res = bass_utils.run_bass_kernel_spmd(nc, per_core,
                                          core_ids=list(range(N_CORES)))
    out = np.concatenate(
        [res.results[c]["out"][inv_perms[c]] for c in range(N_CORES)], axis=0)
    return out.astype(np.float32)
